# revision 16
# baseline (speedup 1.0000x reference)
"""L2 cluster-centroid distance kernel for Trainium2 (8 NeuronCores).

Problem: given embedding [N=1e6, D=128], centers [C=100, D], logits [N, C]:
    assign    = argmax(logits, -1)
    sums      = segment_sum(embedding, assign, C)   # [C, D]
    counts    = segment_sum(1, assign, C)           # [C]
    centroids = sums / max(counts, 1)
    out[c]    = ||centers[c] - centroids[c]||  (0 for empty clusters)

Strategy (data-parallel over N, 8 cores):
  Each core processes ROWS = 124928 rows (976 sub-blocks of 128 rows,
  grouped into chunks so every DMA is >= 0.8 MiB and fully contiguous).
  Both input streams ride ONE SWDGE (gpsimd) DMA queue with an
  fp32 -> bf16 cast in flight, interleaved logits-then-embedding per
  chunk so the one-hot build (Vector: row-max reduce + broadcast
  is_equal, both on bf16) starts as soon as a chunk's logits land.
  Deep bf16 buffer pools (8 chunks ~ 36 MB of HBM runway) keep the DMA
  queue issuing far ahead of the Vector engine, so Vector jitter never
  starves HBM. Per sub-block the segment sums + counts accumulate on
  the Tensor engine into PSUM:
      sums_psum   += onehot.T @ emb      (lhsT = onehot [128, 100])
      counts_psum += onehot.T @ ones
  (A fused rhs=[emb|1] single-matmul variant was tried and rejected:
  the strided SBUF write it needs costs ~8% of DMA stream rate.)
  The one-hot weight matrix is padded to 128 columns (fast weight load);
  its zero padding lives in 3 persistent buffers zeroed once up front.
  Chunk sizes taper at the end so the compute pipeline drains quickly
  after the last DMA byte; the PSUM eviction runs on the otherwise-idle
  Scalar engine. Each core then DMAs its [C, D+1] partial (sums |
  counts) to HBM via the sync HWDGE queue. The host adds the 8 partials
  plus a 576-row tail and does the final (tiny) centroid/distance math.

  bf16 logits can tie at the row max (two fp32 values rounding to the
  same bf16); such rows contribute to both clusters. Measured effect on
  the final distances is ~1e-4 relative — far inside the 2e-2 gate.
"""

import numpy as np

N = 1_000_000
D = 128
C = 100
N_CORES = 8
P = 128            # rows per sub-block == SBUF partitions == matmul K
ROWS = 124928      # rows per core (976 sub-blocks of 128)
N_DEV = N_CORES * ROWS         # 999424 rows on device; tail handled on host

_CACHE = {}

CHUNK_TS = [48] * 19 + [16, 16, 8, 8, 8, 4, 2, 1, 1]


def _build_bass(rows=ROWS, chunk_ts=None, variant="allswdge"):
    import concourse.bacc as bacc
    import concourse.tile as tile
    from concourse import mybir

    if chunk_ts is None:
        chunk_ts = list(CHUNK_TS)
    assert rows == sum(chunk_ts) * P
    tmax = max(chunk_ts)
    nc = bacc.Bacc("TRN2", target_bir_lowering=False, debug=False)
    emb = nc.dram_tensor("embedding", [rows, D], mybir.dt.float32, kind="ExternalInput")
    logit = nc.dram_tensor("logits", [rows, C], mybir.dt.float32, kind="ExternalInput")
    part = nc.dram_tensor("partial", [C, D + 1], mybir.dt.float32, kind="ExternalOutput")

    with tile.TileContext(nc) as tc:
        with (
            tc.tile_pool(name="io", bufs=4) as io_pool,
            tc.tile_pool(name="oh", bufs=3) as oh_pool,
            tc.tile_pool(name="small", bufs=1) as small_pool,
            tc.tile_pool(name="psum", bufs=1, space="PSUM") as psum_pool,
        ):
            NEB = 8  # embedding ring depth (chunks of DMA runway)
            if variant == "merged":
                # rhs = [emb | 1]: one matmul per sub-block produces sums
                # AND counts (psum column D). The ones column lives in 8
                # persistent ring buffers, written once up front; the emb
                # DMA writes only columns [0:D] (256B chunks, 258B pitch).
                ebs = [
                    small_pool.tile(
                        [P, tmax, D + 1], mybir.dt.bfloat16, tag=f"eb{i}", name=f"eb{i}"
                    )
                    for i in range(NEB)
                ]
                for t_ in ebs:
                    nc.vector.memset(t_[:, :, D : D + 1], 1.0)
                ones = None
            else:
                ebs = None
                ones = small_pool.tile([P, 1], mybir.dt.bfloat16)
                nc.vector.memset(ones, 1.0)
            # One-hot is padded M=100 -> 128 (zero columns) so bf16 matmuls
            # get fast-weight-load (needs NumWeights==128). Three persistent
            # buffers, padding zeroed ONCE up front (not per chunk).
            ohs = [
                small_pool.tile(
                    [P, tmax, P], mybir.dt.bfloat16, tag=f"oh{i}", name=f"oh{i}"
                )
                for i in range(3)
            ]
            for t_ in ohs:
                nc.vector.memset(t_[:, :, C:P], 0.0)
            if variant == "merged":
                psum_sums = psum_pool.tile([P, D + 1], mybir.dt.float32)
                psum_cnt = None
            else:
                psum_sums = psum_pool.tile([P, D], mybir.dt.float32)
                psum_cnt = psum_pool.tile([P, 1], mybir.dt.float32)

            off = 0
            for k, t in enumerate(chunk_ts):
                # Row r = off + p*t + n: per (k, p) the t rows are
                # contiguous in HBM -> fully contiguous DMA.
                emb_v = emb[off : off + P * t, :].rearrange("(p n) d -> p n d", n=t)
                log_v = logit[off : off + P * t, :].rearrange("(p n) c -> p n c", n=t)
                off += P * t
                if variant == "merged":
                    lt = io_pool.tile([P, t, C], mybir.dt.bfloat16, tag="log", bufs=8, padded_shape=[P, tmax, C])
                    et = ebs[k % NEB]
                    nc.gpsimd.dma_start(out=lt, in_=log_v)
                    nc.gpsimd.dma_start(out=et[:, 0:t, 0:D], in_=emb_v)
                elif variant == "allswdge":
                    # Both streams on the single SWDGE queue, fp32 -> bf16
                    # cast in flight, logits first so the one-hot build for
                    # chunk k overlaps chunk k's embedding transfer.
                    lt = io_pool.tile([P, t, C], mybir.dt.bfloat16, tag="log", bufs=8, padded_shape=[P, tmax, C])
                    et = io_pool.tile([P, t, D], mybir.dt.bfloat16, tag="emb", bufs=7, padded_shape=[P, tmax, D])
                    nc.gpsimd.dma_start(out=lt, in_=log_v)
                    nc.gpsimd.dma_start(out=et, in_=emb_v)
                else:
                    # two-queue variant: emb via SWDGE (cast), logits via
                    # sync HWDGE as fp32 (Vector compares in fp32).
                    et = io_pool.tile([P, t, D], mybir.dt.bfloat16, tag="emb", bufs=7, padded_shape=[P, tmax, D])
                    lt = io_pool.tile([P, t, C], mybir.dt.float32, tag="log", bufs=6, padded_shape=[P, tmax, C])
                    nc.sync.dma_start(out=lt, in_=log_v)
                    nc.gpsimd.dma_start(out=et, in_=emb_v)

                mx = oh_pool.tile([P, t, 1], lt.dtype, tag="mx", padded_shape=[P, tmax, 1])
                nc.vector.reduce_max(out=mx, in_=lt, axis=mybir.AxisListType.X)
                oh = ohs[k % 3]
                nc.vector.tensor_tensor(
                    out=oh[:, 0:t, 0:C],
                    in0=lt,
                    in1=mx.to_broadcast([P, t, C]),
                    op=mybir.AluOpType.is_equal,
                )
                for n in range(t):
                    first = (k == 0) and (n == 0)
                    last = (k == len(chunk_ts) - 1) and (n == t - 1)
                    if variant == "merged":
                        nc.tensor.matmul(
                            out=psum_sums[:, :],
                            lhsT=oh[:, n, :],
                            rhs=et[:, n, 0 : D + 1],
                            start=first,
                            stop=last,
                            skip_group_check=True,
                        )
                    else:
                        nc.tensor.matmul(
                            out=psum_cnt[:, :],
                            lhsT=oh[:, n, :],
                            rhs=ones[:, :],
                            start=first,
                            stop=last,
                            skip_group_check=True,
                        )
                        nc.tensor.matmul(
                            out=psum_sums[:, :],
                            lhsT=oh[:, n, :],
                            rhs=et[:, n, :],
                            start=first,
                            stop=last,
                            skip_group_check=True,
                        )

            # PSUM eviction on the (idle) Scalar engine: at stream end the
            # Vector engine still has a small one-hot backlog, so keeping
            # the final copies off it shortens the drain.
            outt = small_pool.tile([C, D + 1], mybir.dt.float32)
            if variant == "merged":
                nc.scalar.copy(out=outt[:, :], in_=psum_sums[0:C, :])
            else:
                nc.scalar.copy(out=outt[:, D : D + 1], in_=psum_cnt[0:C, :])
                nc.scalar.copy(out=outt[:, 0:D], in_=psum_sums[0:C, :])
            nc.sync.dma_start(out=part[:, :], in_=outt[:, :])

    nc.compile()
    return nc


def _get_nc():
    if "nc" not in _CACHE:
        _CACHE["nc"] = _build_bass()
    return _CACHE["nc"]


def _finalize(sums, counts, centers):
    centroids = sums / np.maximum(counts, 1.0)[:, None]
    delta = centers.astype(np.float64) - centroids
    sq = np.sum(delta * delta, axis=1)
    dist = np.where(sq > 0, np.sqrt(np.where(sq > 0, sq, 1.0)), 0.0)
    return np.where(counts > 0, dist, 0.0).astype(np.float32)


def kernel(embedding, centers, logits):
    from concourse.bass_utils import run_bass_kernel_spmd

    embedding = np.asarray(embedding, dtype=np.float32)
    centers = np.asarray(centers, dtype=np.float32)
    logits = np.asarray(logits, dtype=np.float32)

    nc = _get_nc()
    in_maps = []
    for c in range(N_CORES):
        lo = c * ROWS
        in_maps.append(
            {
                "embedding": np.ascontiguousarray(embedding[lo : lo + ROWS]),
                "logits": np.ascontiguousarray(logits[lo : lo + ROWS]),
            }
        )
    res = run_bass_kernel_spmd(nc, in_maps, core_ids=list(range(N_CORES)))

    sums = np.zeros((C, D), np.float64)
    counts = np.zeros((C,), np.float64)
    for r in res.results:
        p = r["partial"].astype(np.float64)
        sums += p[:, :D]
        counts += p[:, D]

    # Tail rows the device grid doesn't cover (N - N_DEV = 576 rows).
    te = embedding[N_DEV:]
    tl = logits[N_DEV:]
    if te.shape[0]:
        a = np.argmax(tl, axis=1)
        np.add.at(sums, a, te.astype(np.float64))
        np.add.at(counts, a, 1.0)

    return _finalize(sums, counts, centers)
